# revision 64
# baseline (speedup 1.0000x reference)
"""Trainium2 Bass kernel for nn_LocalFeatureGuided.

Per image (C=128 on partitions, spatial on free dim):
  ingest: x fp32 staged -> xb (bf16, DVE 2x copy) + x8 (fp8e4, Pool copy)
  BN(eval)+GELU on ACT -> zero-padded even/odd column buffers eo (fp8)
  depthwise 7x7 s2 conv entirely on PE: 49 taps as 25 fp8 DoubleRow
    matmuls per 512-col chunk, 2 diag-taps packed per matmul via a
    custom 2-ktile access pattern (0.25 cyc/tap/col)
  t0 (guide) written twice from PSUM: t08 (fp8, scaled 16x) for the
    q0/k0 DR matmuls, t0b (bf16) for the out matmul
  q0 = Wq@t0 (fp8 DR); k_m = Wk@t_m (fp8 DR, x8 tokens); per-channel
    dots <q0,k_m> on DVE (m0-2) + Pool (m3-4), PSUM-consuming
  softmax over 5 logits (mixed fp8 scales folded into a [C,5] descale)
  out = sum_m (Wv^T diag(a_m) Pw^T)^T @ t_m: 5 bf16 matmuls per chunk
    (v-path stays bf16 for precision), cb bias on ACT copy-out
Sharding: data-parallel over batch, 2 images per core, 8 cores.
"""

import os
import numpy as np
from contextlib import ExitStack

import concourse.bass as bass
import concourse.tile as tile
from concourse import bacc, mybir
from concourse import bass_utils
from concourse import tile_utils

alu = mybir.AluOpType
actf = mybir.ActivationFunctionType
F32 = mybir.dt.float32
BF16 = mybir.dt.bfloat16
F8 = mybir.dt.float8e4
DR = mybir.MatmulPerfMode.DoubleRow

B, C, H, W = 16, 128, 128, 128
H2, W2 = H // 2, W // 2
L = H2 * W2            # 4096
NCORES = 8
BPC = B // NCORES      # 2 images per core
EPS = 1e-5
INV_SQRT_C = 1.0 / np.sqrt(128.0)

S_W = 64.0             # conv/qk weight fp8 scale
S_T = 16.0             # t08 (guide token) fp8 scale
S_V = 4096.0           # vts fp8 scale (out path)

SBUF_CAP = 204 * 1024

KSTAGE = int(os.environ.get("KSTAGE", "9"))

EO_P = 134 * 68        # elements per parity plane in eo


def tap_geom(kh, kw):
    """(par, row0, col_off) for tap (kh, kw); elem offset in eo is
    par*EO_P + (kh + 2*h2)*68 + col_off + w2."""
    e = kw - 3
    if e % 2 == 0:
        par, u = 0, e // 2
        off = 1 + u
    else:
        par, u = 1, (e - 1) // 2
        off = 2 + u
    return par, kh, off


def make_pairs():
    """Pair the 49 taps (2 per DR matmul), ordered so the intra-pair
    element-offset delta is >= 0. Returns list of (tapA, tapB, delta)
    with tapB=None for the self-paired leftover (delta 0)."""
    taps = []
    for kh in range(7):
        for kw in range(7):
            par, row0, off = tap_geom(kh, kw)
            eoff = par * EO_P + row0 * 68 + off
            taps.append((eoff, kh, kw))
    taps.sort()
    pairs = []
    for i in range(0, 48, 2):
        ea, _, _ = taps[i]
        eb, _, _ = taps[i + 1]
        pairs.append((taps[i], taps[i + 1], eb - ea))
    pairs.append((taps[48], None, 0))
    return pairs


def build(nc):
    # host-precomputed inputs (see _in_maps): x is the only large tensor
    x_d = nc.dram_tensor("x", (BPC, C, H, W), F32, kind="ExternalInput").ap()
    # vecs cols: 0 bns, 1 bnb, 2 dwb, 3 S_T*dwb, 4 bq, 5 bk, 6 cb, 7:12 mscale
    vecs_d = nc.dram_tensor("vecs", (C, 16), F32, kind="ExternalInput").ap()
    # w8s: [0]=wq8, [1]=wk8, [2+i]=conv pair i   (each [C, 2, 128] fp8)
    w8s_d = nc.dram_tensor("w8s", (C, 27, 2, C), F8, kind="ExternalInput").ap()
    wvb_d = nc.dram_tensor("wvb", (C, C), BF16, kind="ExternalInput").ap()
    pwt_d = nc.dram_tensor("pwt", (C, C), F32, kind="ExternalInput").ap()
    out_d = nc.dram_tensor("out", (BPC, C, H2, W2), F32, kind="ExternalOutput").ap()

    pairs = make_pairs()

    with tile.TileContext(nc) as tc, ExitStack() as ctx:
        tp = lambda name, bufs, **kw: ctx.enter_context(
            tc.tile_pool(name=name, bufs=bufs, **kw))

        wpool = tp("weights", 1)     # persistent weights + eo
        stgp = tp("stg", 4)          # [C,2048] f32 staging
        x8p = tp("x8", 2)            # [C,16384] fp8
        r8p = tp("r8", 2)            # [C,16384] fp8 residual x - fp8(x)
        t08p = tp("t08", 2)          # [C,4096] fp8
        q0p = tp("q0", 1)            # [C,4096] bf16
        scrp = tp("scr", 1)          # [C,1024] bf16 dot scratch
        # (scr tiles are write-only scratch for dot accum ops)
        vtp = tp("vt", 10)           # vts fp8 pair tiles [C,2,C]
        outp = tp("outc", 3)
        vecp = tp("vec", 16)
        emp = tp("em", 4)
        pp512 = tp("pp512", 3, space="PSUM")
        ppk = tp("ppk", 2, space="PSUM")    # [C,1024]
        ppw = tp("ppw", 1, space="PSUM")    # [C,128]

        # ---------- phase 0: 4 weight DMA loads ----------
        vecs = wpool.tile([C, 16], F32)
        nc.scalar.dma_start(vecs[:], vecs_d)
        w8s = wpool.tile([C, 27, 2, C], F8)
        nc.gpsimd.dma_start(w8s[:], w8s_d)
        wv_b16 = wpool.tile([C, C], BF16)
        nc.gpsimd.dma_start(wv_b16[:], wvb_d)
        pwT = wpool.tile([C, C], F32)
        nc.gpsimd.dma_start(pwT[:], pwt_d)

        bns = vecs[:, 0:1]
        bnb = vecs[:, 1:2]
        dwb = vecs[:, 2:3]
        dwbS = vecs[:, 3:4]
        bq = vecs[:, 4:5]
        bk = vecs[:, 5:6]
        cb = vecs[:, 6:7]
        mscale = vecs[:, 7:12]
        wq8 = w8s[:, 0]
        wk8 = w8s[:, 1]
        conv8 = [w8s[:, 2 + i] for i in range(25)]

        # eo buffer (persistent; pads memset once)
        eo = wpool.tile([C, 2, 134, 68], F8)
        nc.gpsimd.memset(eo[:, :, 0:3], 0.0)
        nc.gpsimd.memset(eo[:, :, 131:134], 0.0)
        nc.vector.memset(eo[:, 0, 3:131, 0:1], 0.0)
        nc.vector.memset(eo[:, 0, 3:131, 65:68], 0.0)
        nc.vector.memset(eo[:, 1, 3:131, 0:2], 0.0)
        nc.vector.memset(eo[:, 1, 3:131, 66:68], 0.0)

        def g_ap(kh, kw, a, b):
            par, row0, off = tap_geom(kh, kw)
            return eo[:, par, row0 + 2 * a:row0 + 2 * b:2, off:off + 64]

        def pair_rhs(pi, a, nr=8):
            """rhs [C, 2, nr, 64] for pair pi at h2 rows [a, a+nr)."""
            (eA, khA, kwA), tb, delta = pairs[pi]
            v = g_ap(khA, kwA, a, a + nr)
            ap = [list(v.ap[0]), [delta, 2], list(v.ap[1]), list(v.ap[2])]
            return bass.AP(v.tensor, v.offset, ap)

        def dup2(v):
            """Duplicate a [C, N...] view along a stride-0 ktile dim."""
            ap = [list(v.ap[0]), [0, 2]] + [list(d) for d in v.ap[1:]]
            return bass.AP(v.tensor, v.offset, ap)

        # ---------- per image state ----------
        imgs = []
        for img in range(BPC):
            st = {}
            st["x8"] = x8p.tile([C, H * W], F8, tag="x8", name=f"x8_{img}")
            st["r8"] = r8p.tile([C, H * W], F8, tag="r8", name=f"r8_{img}")
            st["t08"] = t08p.tile([C, L], F8, tag="t08", name=f"t08_{img}")
            st["q0"] = q0p.tile([C, L], F32, tag="q0", name=f"q0_{img}")
            st["q0s"] = vecp.tile([C, 8], F32, tag="q0s", name=f"q0s{img}")
            st["dots"] = vecp.tile([C, 20], F32, tag="dots", name=f"dots{img}")
            imgs.append(st)

        def tok8(st, m, hf, j):
            """fp8 token view [C, 2(ktile), 8, 64] for k matmul of
            512 cols at l = hf*1024 + j*512."""
            h2lo = hf * 16 + j * 8
            if m == 0:
                c0 = h2lo * 64
                return dup2(st["t08"][:, c0:c0 + 512])
            p, q = (m - 1) // 2, (m - 1) % 2
            x83 = st["x8"][:].rearrange("c (h w) -> c h w", h=H)
            v = x83[:, 2 * h2lo + p:min(2 * (h2lo + 8) + p, H):2, q::2]
            return dup2(v)

        def tokpair(st, which, p, ch):
            """fp8 token pair view [C, 2(m ktile), 8, 64] for out matmul
            chunk ch: ktile j = token (p, q=j), from x8 or r8."""
            h2lo = ch * 8
            t3 = st[which][:].rearrange("c (h w) -> c h w", h=H)
            v = t3[:, 2 * h2lo + p:min(2 * (h2lo + 8) + p, H):2, 0::2]
            ap = [list(v.ap[0]), [1, 2], list(v.ap[1]), list(v.ap[2])]
            return bass.AP(v.tensor, v.offset, ap)

        def ingest_band(img, s):
            """DMA band s (16 rows / 2048 cols), convert, GELU into eo."""
            st = imgs[img]
            xi = x_d[img].rearrange("c h w -> c (h w)")
            sl = slice(s * 2048, (s + 1) * 2048)
            stg = stgp.tile([C, 2048], F32, tag="st", name="stg")
            nc.sync.dma_start(stg[:], xi[:, sl])
            ieng = nc.vector if (img == 0 and s < 2) else nc.gpsimd
            ieng.tensor_copy(st["x8"][:, sl], stg[:])
            ieng.tensor_tensor(st["r8"][:, sl], stg[:], st["x8"][:, sl],
                               alu.subtract)
            r0 = 16 * s
            # one ACT op for both parities: out dims [2(par),16,64], the
            # even plane lands at col 1, odd at col 2 (offset EO_P+1 later)
            ov = eo[:, 0, 3 + r0:3 + r0 + 16, 1:65]
            oap = bass.AP(ov.tensor, ov.offset,
                          [list(ov.ap[0]), [EO_P + 1, 2],
                           list(ov.ap[1]), list(ov.ap[2])])
            iv = stg[:]
            iap = bass.AP(iv.tensor, iv.offset,
                          [list(iv.ap[0]), [1, 2], [128, 16], [2, 64]])
            nc.scalar.activation(oap, iap, actf.Gelu,
                                 bias=bnb[:, 0:1], scale=bns[:, 0:1])

        def conv_q0(img, hf):
            """conv + t0 copies + q0 for one half (1024 cols); q0 matmuls
            issued after both conv chunks so they never stall the PE."""
            st = imgs[img]
            fine = False
            for j in range(2):
                ch = hf * 2 + j
                a = ch * 8
                sl = slice(ch * 512, (ch + 1) * 512)
                ps = pp512.tile([C, 512], F32, tag="ps")
                if fine:
                    for h in range(2):
                        reg = ps[:, h * 256:(h + 1) * 256]
                        for pi in range(25):
                            nc.tensor.matmul(
                                reg, conv8[pi][:],
                                pair_rhs(pi, a + 4 * h, nr=4),
                                start=(pi == 0), stop=(pi == 24),
                                perf_mode=DR)
                else:
                    for pi in range(25):
                        nc.tensor.matmul(ps[:], conv8[pi][:], pair_rhs(pi, a),
                                         start=(pi == 0), stop=(pi == 24),
                                         perf_mode=DR)
                nc.scalar.activation(st["t08"][:, sl], ps[:],
                                     actf.Identity, bias=dwbS[:, 0:1],
                                     scale=S_T / S_W)
            for j in range(2):
                ch = hf * 2 + j
                sl = slice(ch * 512, (ch + 1) * 512)
                qps = pp512.tile([C, 512], F32, tag="ps", name="qps")
                nc.tensor.matmul(qps[:], wq8[:], dup2(st["t08"][:, sl]),
                                 start=True, stop=True, perf_mode=DR)
                if img == 0:
                    nc.vector.tensor_scalar(st["q0"][:, sl], qps[:],
                                            1.0 / (S_W * S_T), bq[:, 0:1],
                                            alu.mult, alu.add,
                                            accum_out=st["q0s"][:, ch:ch + 1])
                else:
                    nc.scalar.activation(st["q0"][:, sl], qps[:],
                                         actf.Identity, bias=bq[:, 0:1],
                                         scale=1.0 / (S_W * S_T),
                                         accum_out=st["q0s"][:, ch:ch + 1])

        def k_dots(img, hf):
            """k matmuls + per-channel dots for one half (1024 cols)."""
            st = imgs[img]
            q0h = st["q0"][:, hf * 1024:(hf + 1) * 1024]
            for m in range(5):
                kp = ppk.tile([C, 1024], F32, name="kp")
                for j in range(2):
                    nc.tensor.matmul(kp[:, j * 512:(j + 1) * 512],
                                     wk8[:], tok8(st, m, hf, j),
                                     start=True, stop=True, perf_mode=DR)
                scr = scrp.tile([C, 1024], BF16, tag="s", name="scr")
                nc.vector.scalar_tensor_tensor(
                    scr[:], q0h, 1.0, kp[:], alu.mult, alu.mult,
                    accum_out=st["dots"][:, m * 4 + hf:m * 4 + hf + 1])

        def softmax_vts(img):
            st = imgs[img]
            # s5 = descale(sum_hf dots) + bk*sum(q0)
            s5 = vecp.tile([C, 5], F32, tag="s5")
            nc.vector.tensor_reduce(
                s5[:], st["dots"][:].rearrange("c (m h) -> c m h", m=5),
                mybir.AxisListType.X, alu.add)
            nc.vector.tensor_tensor(s5[:], s5[:], mscale, alu.mult)
            q0sum = vecp.tile([C, 1], F32, tag="v")
            nc.vector.tensor_reduce(q0sum[:], st["q0s"][:],
                                    mybir.AxisListType.X, alu.add)
            bkq = vecp.tile([C, 1], F32, tag="v")
            nc.vector.tensor_tensor(bkq[:], bk, q0sum[:], alu.mult)
            nc.vector.tensor_tensor(s5[:], s5[:],
                                    bkq[:, 0:1].broadcast_to((C, 5)), alu.add)
            mx = vecp.tile([C, 1], F32, tag="v")
            nc.vector.tensor_reduce(mx[:], s5[:], mybir.AxisListType.X, alu.max)
            nmx = vecp.tile([C, 1], F32, tag="v")
            nc.vector.tensor_scalar_mul(nmx[:], mx[:], -INV_SQRT_C)
            e5 = vecp.tile([C, 5], F32, tag="s5")
            nc.scalar.activation(e5[:], s5[:], actf.Exp, bias=nmx[:, 0:1],
                                 scale=INV_SQRT_C)
            ssum = vecp.tile([C, 1], F32, tag="v")
            nc.vector.tensor_reduce(ssum[:], e5[:], mybir.AxisListType.X,
                                    alu.add)
            sinv = vecp.tile([C, 1], F32, tag="v")
            nc.vector.reciprocal(sinv[:], ssum[:])
            a5 = vecp.tile([C, 5], F32, tag="s5")
            nc.vector.tensor_scalar_mul(a5[:], e5[:], sinv[:, 0:1])

            # vts_m = Wv^T diag(a_m) Pw^T as fp8 value+residual pair tiles:
            # vt0 = [S_V/S_T * vts0 ; its fp8 residual]
            # vt12/vt34 = [S_V*vts_m ; S_V*vts_m'] (m pairs), vtr12/vtr34 =
            # matching fp8 residuals. All for DoubleRow out matmuls.
            names = ["vt0", "vt12", "vt34", "vtr12", "vtr34"]
            tiles = {}
            for nmv in names:
                tiles[nmv] = vtp.tile([C, 2, C], F8, tag="vt", name=f"{nmv}_{img}")
            slots = []
            for m in range(5):
                if m == 0:
                    slots.append((S_V / S_T, tiles["vt0"][:, 0],
                                  tiles["vt0"][:, 1]))
                else:
                    main = "vt12" if m <= 2 else "vt34"
                    resid = "vtr12" if m <= 2 else "vtr34"
                    h = (m - 1) % 2
                    slots.append((S_V, tiles[main][:, h], tiles[resid][:, h]))
            # stage-batched so em (DVE) / vp (PE) / main+resid pipeline
            ems, vps = [], []
            for m in range(5):
                em = emp.tile([C, C], BF16, tag="em")
                nc.vector.tensor_scalar_mul(em[:], pwT[:], a5[:, m:m + 1])
                ems.append(em)
            for m in range(5):
                vp = ppw.tile([C, C], F32, tag="w")
                nc.tensor.matmul(vp[:], wv_b16[:], ems[m][:], start=True,
                                 stop=True)
                vps.append(vp)
                sc8, mt, rt = slots[m]
                if img == 0:
                    nc.vector.tensor_scalar_mul(mt, vp[:], sc8)
                else:
                    nc.scalar.activation(mt, vp[:], actf.Identity,
                                         bias=0.0, scale=sc8)
                nc.vector.scalar_tensor_tensor(rt, vp[:], sc8, mt,
                                               alu.mult, alu.subtract)
            st["vts"] = tiles

        def phase_b(img, c0=0, c1=8, wide=False):
            st = imgs[img]
            v = st["vts"]
            oi = out_d[img].rearrange("c h w -> c (h w)")

            def out_mms(ps, ch):
                c0_ = ch * 512
                mm = nc.tensor.matmul
                mm(ps, v["vt0"][:], dup2(st["t08"][:, c0_:c0_ + 512]),
                   start=True, stop=False, perf_mode=DR)
                mm(ps, v["vt12"][:], tokpair(st, "x8", 0, ch),
                   start=False, stop=False, perf_mode=DR)
                mm(ps, v["vt34"][:], tokpair(st, "x8", 1, ch),
                   start=False, stop=False, perf_mode=DR)
                mm(ps, v["vtr12"][:], tokpair(st, "x8", 0, ch),
                   start=False, stop=False, perf_mode=DR)
                mm(ps, v["vtr34"][:], tokpair(st, "x8", 1, ch),
                   start=False, stop=False, perf_mode=DR)
                mm(ps, v["vt12"][:], tokpair(st, "r8", 0, ch),
                   start=False, stop=False, perf_mode=DR)
                mm(ps, v["vt34"][:], tokpair(st, "r8", 1, ch),
                   start=False, stop=True, perf_mode=DR)

            if wide:
                # B1 uses the ppk banks (free after the last k matmul) for a
                # deeper psum rotation; copies/DMAs stay at 512 granularity
                for cp in range(c0 // 2, c1 // 2):
                    ps = ppk.tile([C, 1024], F32, name="kp")
                    for h in range(2):
                        out_mms(ps[:, h * 512:(h + 1) * 512], cp * 2 + h)
                    for h in range(2):
                        ch = cp * 2 + h
                        oc = outp.tile([C, 512], F32, tag="oc")
                        nc.scalar.activation(oc[:], ps[:, h * 512:(h + 1) * 512],
                                             actf.Identity, bias=cb[:, 0:1],
                                             scale=1.0 / S_V)
                        nc.sync.dma_start(oi[:, ch * 512:(ch + 1) * 512], oc[:])
                return
            for ch in range(c0, c1):
                ps = pp512.tile([C, 512], F32, tag="ps")
                out_mms(ps[:], ch)
                oc = outp.tile([C, 512], F32, tag="oc")
                nc.scalar.activation(oc[:], ps[:], actf.Identity,
                                     bias=cb[:, 0:1], scale=1.0 / S_V)
                nc.sync.dma_start(oi[:, ch * 512:(ch + 1) * 512], oc[:])

        def stage_dump(img):
            st = imgs[img]
            oi = out_d[img].rearrange("c h w -> c (h w)")
            tmp = q0p.tile([C, L], F32, tag="dump")
            if KSTAGE == 2:
                nc.scalar.activation(tmp[:], st["t08"][:], actf.Identity,
                                     bias=0.0, scale=1.0 / S_T)
                nc.sync.dma_start(oi, tmp[:])
            elif KSTAGE == 4:
                nc.sync.dma_start(oi, st["q0"][:])

        if KSTAGE >= 9:
            # global software pipeline over 16 ingest bands; conv half hf
            # of an image needs that image's bands <= 2*hf+2; k/dots of an
            # hf issue after the next hf's conv so dot drains hide under it
            for s in range(3):
                ingest_band(0, s, sub=True)
            conv_q0(0, 0); ingest_band(0, 3); ingest_band(0, 4)
            conv_q0(0, 1); k_dots(0, 0); ingest_band(0, 5); ingest_band(0, 6)
            conv_q0(0, 2); k_dots(0, 1); ingest_band(0, 7); ingest_band(1, 0)
            conv_q0(0, 3); k_dots(0, 2); ingest_band(1, 1); ingest_band(1, 2)
            k_dots(0, 3)
            conv_q0(1, 0); ingest_band(1, 3); ingest_band(1, 4)
            conv_q0(1, 1); k_dots(1, 0); ingest_band(1, 5); ingest_band(1, 6)
            conv_q0(1, 2); k_dots(1, 1); ingest_band(1, 7)
            softmax_vts(0)
            conv_q0(1, 3); k_dots(1, 2)
            phase_b(0, 0, 2)
            k_dots(1, 3)
            phase_b(0, 2, 8)
            softmax_vts(1)
            phase_b(1)
        else:
            for img in range(BPC):
                for s in range(8):
                    ingest_band(img, s)
                for hf in range(4):
                    conv_q0(img, hf)
                for hf in range(4):
                    k_dots(img, hf)
                stage_dump(img)
    return nc


_CACHE = {}


def _get_nc():
    if "nc" not in _CACHE:
        tile_utils.max_sbuf_usage = SBUF_CAP
        nc = bacc.Bacc("TRN2", target_bir_lowering=False, debug=False,
                       num_devices=NCORES)
        build(nc)
        nc.compile()
        _CACHE["nc"] = nc
    return _CACHE["nc"]


def _in_maps(inputs):
    import ml_dtypes
    f8 = ml_dtypes.float8_e4m3
    b16 = ml_dtypes.bfloat16

    gam = np.asarray(inputs["bn_gamma"], np.float32).reshape(C)
    bet = np.asarray(inputs["bn_beta"], np.float32).reshape(C)
    mea = np.asarray(inputs["bn_mean"], np.float32).reshape(C)
    var = np.asarray(inputs["bn_var"], np.float32).reshape(C)
    dww = np.asarray(inputs["dw_w"], np.float32).reshape(C, 49)
    dwb = np.asarray(inputs["dw_b"], np.float32).reshape(C)
    qkv_w = np.asarray(inputs["qkv_w"], np.float32).reshape(3 * C, C)
    qkv_b = np.asarray(inputs["qkv_b"], np.float32).reshape(3 * C)
    pw = np.asarray(inputs["proj_w"], np.float32).reshape(C, C)
    pb = np.asarray(inputs["proj_b"], np.float32).reshape(C)

    bns = gam / np.sqrt(var + EPS)
    bnb = bet - mea * bns
    bq, bk, bv = qkv_b[0:C], qkv_b[C:2 * C], qkv_b[2 * C:3 * C]
    cb = pw @ bv + pb

    vecs = np.zeros((C, 16), np.float32)
    vecs[:, 0] = bns
    vecs[:, 1] = bnb
    vecs[:, 2] = dwb
    vecs[:, 3] = S_T * dwb
    vecs[:, 4] = bq
    vecs[:, 5] = bk
    vecs[:, 6] = cb
    vecs[:, 7] = 1.0 / (S_W * S_T)
    vecs[:, 8:12] = 1.0 / S_W

    w8s = np.zeros((C, 27, 2, C), np.float32)
    wqT = qkv_w[0:C].T          # [j, cq]
    wkT = qkv_w[C:2 * C].T
    w8s[:, 0, 0] = (S_W / 2) * wqT
    w8s[:, 0, 1] = (S_W / 2) * wqT
    w8s[:, 1, 0] = (S_W / 2) * wkT
    w8s[:, 1, 1] = (S_W / 2) * wkT
    ii = np.arange(C)
    for i, (ta, tb, delta) in enumerate(make_pairs()):
        _, khA, kwA = ta
        if tb is None:
            w8s[ii, 2 + i, 0, ii] = (S_W / 2) * dww[:, khA * 7 + kwA]
            w8s[ii, 2 + i, 1, ii] = (S_W / 2) * dww[:, khA * 7 + kwA]
        else:
            _, khB, kwB = tb
            w8s[ii, 2 + i, 0, ii] = S_W * dww[:, khA * 7 + kwA]
            w8s[ii, 2 + i, 1, ii] = S_W * dww[:, khB * 7 + kwB]

    shared = {
        "vecs": vecs,
        "w8s": w8s.astype(f8),
        "wvb": qkv_w[2 * C:3 * C].astype(b16),
        "pwt": np.ascontiguousarray(pw.T),
    }
    xf = np.ascontiguousarray(np.asarray(inputs["x"], np.float32))
    return [dict(shared, x=xf[i * BPC:(i + 1) * BPC]) for i in range(NCORES)]


def kernel(x, bn_gamma, bn_beta, bn_mean, bn_var, dw_w, dw_b, qkv_w, qkv_b,
           proj_w, proj_b):
    nc = _get_nc()
    in_maps = _in_maps(dict(
        x=x, bn_gamma=bn_gamma, bn_beta=bn_beta, bn_mean=bn_mean,
        bn_var=bn_var, dw_w=dw_w, dw_b=dw_b, qkv_w=qkv_w, qkv_b=qkv_b,
        proj_w=proj_w, proj_b=proj_b))
    res = bass_utils.run_bass_kernel_spmd(nc, in_maps,
                                          core_ids=list(range(NCORES)))
    return np.concatenate([r["out"] for r in res.results], axis=0)


# revision 65
# speedup vs baseline: 1.0101x; 1.0101x over previous
"""Trainium2 Bass kernel for nn_LocalFeatureGuided.

Per image (C=128 on partitions, spatial on free dim):
  ingest: x fp32 staged -> xb (bf16, DVE 2x copy) + x8 (fp8e4, Pool copy)
  BN(eval)+GELU on ACT -> zero-padded even/odd column buffers eo (fp8)
  depthwise 7x7 s2 conv entirely on PE: 49 taps as 25 fp8 DoubleRow
    matmuls per 512-col chunk, 2 diag-taps packed per matmul via a
    custom 2-ktile access pattern (0.25 cyc/tap/col)
  t0 (guide) written twice from PSUM: t08 (fp8, scaled 16x) for the
    q0/k0 DR matmuls, t0b (bf16) for the out matmul
  q0 = Wq@t0 (fp8 DR); k_m = Wk@t_m (fp8 DR, x8 tokens); per-channel
    dots <q0,k_m> on DVE (m0-2) + Pool (m3-4), PSUM-consuming
  softmax over 5 logits (mixed fp8 scales folded into a [C,5] descale)
  out = sum_m (Wv^T diag(a_m) Pw^T)^T @ t_m: 5 bf16 matmuls per chunk
    (v-path stays bf16 for precision), cb bias on ACT copy-out
Sharding: data-parallel over batch, 2 images per core, 8 cores.
"""

import os
import numpy as np
from contextlib import ExitStack

import concourse.bass as bass
import concourse.tile as tile
from concourse import bacc, mybir
from concourse import bass_utils
from concourse import tile_utils

alu = mybir.AluOpType
actf = mybir.ActivationFunctionType
F32 = mybir.dt.float32
BF16 = mybir.dt.bfloat16
F8 = mybir.dt.float8e4
DR = mybir.MatmulPerfMode.DoubleRow

B, C, H, W = 16, 128, 128, 128
H2, W2 = H // 2, W // 2
L = H2 * W2            # 4096
NCORES = 8
BPC = B // NCORES      # 2 images per core
EPS = 1e-5
INV_SQRT_C = 1.0 / np.sqrt(128.0)

S_W = 64.0             # conv/qk weight fp8 scale
S_T = 16.0             # t08 (guide token) fp8 scale
S_V0 = 256.0           # m0 vts fp8 scale; out group scale = S_V0*S_T
S_V = 4096.0           # vts fp8 scale (out path)

SBUF_CAP = 204 * 1024

KSTAGE = int(os.environ.get("KSTAGE", "9"))

EO_P = 134 * 68        # elements per parity plane in eo


def tap_geom(kh, kw):
    """(par, row0, col_off) for tap (kh, kw); elem offset in eo is
    par*EO_P + (kh + 2*h2)*68 + col_off + w2."""
    e = kw - 3
    if e % 2 == 0:
        par, u = 0, e // 2
        off = 1 + u
    else:
        par, u = 1, (e - 1) // 2
        off = 2 + u
    return par, kh, off


def make_pairs():
    """Pair the 49 taps (2 per DR matmul), ordered so the intra-pair
    element-offset delta is >= 0. Returns list of (tapA, tapB, delta)
    with tapB=None for the self-paired leftover (delta 0)."""
    taps = []
    for kh in range(7):
        for kw in range(7):
            par, row0, off = tap_geom(kh, kw)
            eoff = par * EO_P + row0 * 68 + off
            taps.append((eoff, kh, kw))
    taps.sort()
    pairs = []
    for i in range(0, 48, 2):
        ea, _, _ = taps[i]
        eb, _, _ = taps[i + 1]
        pairs.append((taps[i], taps[i + 1], eb - ea))
    pairs.append((taps[48], None, 0))
    return pairs


def build(nc):
    # host-precomputed inputs (see _in_maps): x is the only large tensor
    x_d = nc.dram_tensor("x", (BPC, C, H, W), F32, kind="ExternalInput").ap()
    # vecs cols: 0 bns, 1 bnb, 2 dwb, 3 S_T*dwb, 4 bq, 5 bk, 6 cb, 7:12 mscale
    vecs_d = nc.dram_tensor("vecs", (C, 16), F32, kind="ExternalInput").ap()
    # w8s: [0]=wq8, [1]=wk8, [2+i]=conv pair i   (each [C, 2, 128] fp8)
    w8s_d = nc.dram_tensor("w8s", (C, 27, 2, C), F8, kind="ExternalInput").ap()
    wvb_d = nc.dram_tensor("wvb", (C, C), BF16, kind="ExternalInput").ap()
    pwt_d = nc.dram_tensor("pwt", (C, C), F32, kind="ExternalInput").ap()
    out_d = nc.dram_tensor("out", (BPC, C, H2, W2), F32, kind="ExternalOutput").ap()

    pairs = make_pairs()

    with tile.TileContext(nc) as tc, ExitStack() as ctx:
        tp = lambda name, bufs, **kw: ctx.enter_context(
            tc.tile_pool(name=name, bufs=bufs, **kw))

        wpool = tp("weights", 1)     # persistent weights + eo
        stgp = tp("stg", 4)          # [C,2048] f32 staging
        x8p = tp("x8", 2)            # [C,16384] fp8
        r8p = tp("r8", 2)            # [C,16384] fp8 residual x - fp8(x)
        t08p = tp("t08", 2)          # [C,4096] fp8
        q0p = tp("q0", 1)            # [C,4096] bf16
        scrp = tp("scr", 1)          # [C,1024] bf16 dot scratch
        # (scr tiles are write-only scratch for dot accum ops)
        vtp = tp("vt", 10)           # vts fp8 pair tiles [C,2,C]
        outp = tp("outc", 3)
        vecp = tp("vec", 16)
        emp = tp("em", 4)
        pp512 = tp("pp512", 3, space="PSUM")
        ppk = tp("ppk", 2, space="PSUM")    # [C,1024]
        ppw = tp("ppw", 1, space="PSUM")    # [C,128]

        # ---------- phase 0: 4 weight DMA loads ----------
        vecs = wpool.tile([C, 16], F32)
        nc.scalar.dma_start(vecs[:], vecs_d)
        w8s = wpool.tile([C, 27, 2, C], F8)
        nc.gpsimd.dma_start(w8s[:], w8s_d)
        wv_b16 = wpool.tile([C, C], BF16)
        nc.gpsimd.dma_start(wv_b16[:], wvb_d)
        pwT = wpool.tile([C, C], F32)
        nc.gpsimd.dma_start(pwT[:], pwt_d)

        bns = vecs[:, 0:1]
        bnb = vecs[:, 1:2]
        dwb = vecs[:, 2:3]
        dwbS = vecs[:, 3:4]
        bq = vecs[:, 4:5]
        bk = vecs[:, 5:6]
        cb = vecs[:, 6:7]
        mscale = vecs[:, 7:12]
        wq8 = w8s[:, 0]
        wk8 = w8s[:, 1]
        conv8 = [w8s[:, 2 + i] for i in range(25)]

        # eo buffer (persistent; pads memset once)
        eo = wpool.tile([C, 2, 134, 68], F8)
        nc.gpsimd.memset(eo[:, :, 0:3], 0.0)
        nc.gpsimd.memset(eo[:, :, 131:134], 0.0)
        nc.vector.memset(eo[:, 0, 3:131, 0:1], 0.0)
        nc.vector.memset(eo[:, 0, 3:131, 65:68], 0.0)
        nc.vector.memset(eo[:, 1, 3:131, 0:2], 0.0)
        nc.vector.memset(eo[:, 1, 3:131, 66:68], 0.0)

        def g_ap(kh, kw, a, b):
            par, row0, off = tap_geom(kh, kw)
            return eo[:, par, row0 + 2 * a:row0 + 2 * b:2, off:off + 64]

        def pair_rhs(pi, a, nr=8):
            """rhs [C, 2, nr, 64] for pair pi at h2 rows [a, a+nr)."""
            (eA, khA, kwA), tb, delta = pairs[pi]
            v = g_ap(khA, kwA, a, a + nr)
            ap = [list(v.ap[0]), [delta, 2], list(v.ap[1]), list(v.ap[2])]
            return bass.AP(v.tensor, v.offset, ap)

        def dup2(v):
            """Duplicate a [C, N...] view along a stride-0 ktile dim."""
            ap = [list(v.ap[0]), [0, 2]] + [list(d) for d in v.ap[1:]]
            return bass.AP(v.tensor, v.offset, ap)

        # ---------- per image state ----------
        imgs = []
        for img in range(BPC):
            st = {}
            st["x8"] = x8p.tile([C, H * W], F8, tag="x8", name=f"x8_{img}")
            st["r8"] = r8p.tile([C, H * W], F8, tag="r8", name=f"r8_{img}")
            st["t08"] = t08p.tile([C, L], F8, tag="t08", name=f"t08_{img}")
            st["q0"] = q0p.tile([C, L], F32, tag="q0", name=f"q0_{img}")
            st["q0s"] = vecp.tile([C, 8], F32, tag="q0s", name=f"q0s{img}")
            st["dots"] = vecp.tile([C, 20], F32, tag="dots", name=f"dots{img}")
            imgs.append(st)

        def tok8(st, m, hf, j):
            """fp8 token view [C, 2(ktile), 8, 64] for k matmul of
            512 cols at l = hf*1024 + j*512."""
            h2lo = hf * 16 + j * 8
            if m == 0:
                c0 = h2lo * 64
                return dup2(st["t08"][:, c0:c0 + 512])
            p, q = (m - 1) // 2, (m - 1) % 2
            x83 = st["x8"][:].rearrange("c (h w) -> c h w", h=H)
            v = x83[:, 2 * h2lo + p:min(2 * (h2lo + 8) + p, H):2, q::2]
            return dup2(v)

        def tokpair(st, which, p, ch):
            """fp8 token pair view [C, 2(m ktile), 8, 64] for out matmul
            chunk ch: ktile j = token (p, q=j), from x8 or r8."""
            h2lo = ch * 8
            t3 = st[which][:].rearrange("c (h w) -> c h w", h=H)
            v = t3[:, 2 * h2lo + p:min(2 * (h2lo + 8) + p, H):2, 0::2]
            ap = [list(v.ap[0]), [1, 2], list(v.ap[1]), list(v.ap[2])]
            return bass.AP(v.tensor, v.offset, ap)

        def ingest_band(img, s):
            """DMA band s (16 rows / 2048 cols), convert, GELU into eo."""
            st = imgs[img]
            xi = x_d[img].rearrange("c h w -> c (h w)")
            sl = slice(s * 2048, (s + 1) * 2048)
            stg = stgp.tile([C, 2048], F32, tag="st", name="stg")
            nc.sync.dma_start(stg[:], xi[:, sl])
            ieng = nc.vector if (img == 0 and s < 2) else nc.gpsimd
            ieng.tensor_copy(st["x8"][:, sl], stg[:])
            ieng.tensor_tensor(st["r8"][:, sl], stg[:], st["x8"][:, sl],
                               alu.subtract)
            r0 = 16 * s
            # one ACT op for both parities: out dims [2(par),16,64], the
            # even plane lands at col 1, odd at col 2 (offset EO_P+1 later)
            ov = eo[:, 0, 3 + r0:3 + r0 + 16, 1:65]
            oap = bass.AP(ov.tensor, ov.offset,
                          [list(ov.ap[0]), [EO_P + 1, 2],
                           list(ov.ap[1]), list(ov.ap[2])])
            iv = stg[:]
            iap = bass.AP(iv.tensor, iv.offset,
                          [list(iv.ap[0]), [1, 2], [128, 16], [2, 64]])
            nc.scalar.activation(oap, iap, actf.Gelu,
                                 bias=bnb[:, 0:1], scale=bns[:, 0:1])

        def conv_q0(img, hf):
            """conv + t0 copies + q0 for one half (1024 cols); q0 matmuls
            issued after both conv chunks so they never stall the PE."""
            st = imgs[img]
            fine = False
            for j in range(2):
                ch = hf * 2 + j
                a = ch * 8
                sl = slice(ch * 512, (ch + 1) * 512)
                ps = pp512.tile([C, 512], F32, tag="ps")
                if fine:
                    for h in range(2):
                        reg = ps[:, h * 256:(h + 1) * 256]
                        for pi in range(25):
                            nc.tensor.matmul(
                                reg, conv8[pi][:],
                                pair_rhs(pi, a + 4 * h, nr=4),
                                start=(pi == 0), stop=(pi == 24),
                                perf_mode=DR)
                else:
                    for pi in range(25):
                        nc.tensor.matmul(ps[:], conv8[pi][:], pair_rhs(pi, a),
                                         start=(pi == 0), stop=(pi == 24),
                                         perf_mode=DR)
                nc.scalar.activation(st["t08"][:, sl], ps[:],
                                     actf.Identity, bias=dwbS[:, 0:1],
                                     scale=S_T / S_W)
            for j in range(2):
                ch = hf * 2 + j
                sl = slice(ch * 512, (ch + 1) * 512)
                qps = pp512.tile([C, 512], F32, tag="ps", name="qps")
                nc.tensor.matmul(qps[:], wq8[:], dup2(st["t08"][:, sl]),
                                 start=True, stop=True, perf_mode=DR)
                if img == 0:
                    nc.vector.tensor_scalar(st["q0"][:, sl], qps[:],
                                            1.0 / (S_W * S_T), bq[:, 0:1],
                                            alu.mult, alu.add,
                                            accum_out=st["q0s"][:, ch:ch + 1])
                else:
                    nc.scalar.activation(st["q0"][:, sl], qps[:],
                                         actf.Identity, bias=bq[:, 0:1],
                                         scale=1.0 / (S_W * S_T),
                                         accum_out=st["q0s"][:, ch:ch + 1])

        def k_dots(img, hf):
            """k matmuls + per-channel dots for one half (1024 cols)."""
            st = imgs[img]
            q0h = st["q0"][:, hf * 1024:(hf + 1) * 1024]
            for m in range(5):
                kp = ppk.tile([C, 1024], F32, name="kp")
                for j in range(2):
                    nc.tensor.matmul(kp[:, j * 512:(j + 1) * 512],
                                     wk8[:], tok8(st, m, hf, j),
                                     start=True, stop=True, perf_mode=DR)
                scr = scrp.tile([C, 1024], BF16, tag="s", name="scr")
                nc.vector.scalar_tensor_tensor(
                    scr[:], q0h, 1.0, kp[:], alu.mult, alu.mult,
                    accum_out=st["dots"][:, m * 4 + hf:m * 4 + hf + 1])

        def softmax_vts(img):
            st = imgs[img]
            # s5 = descale(sum_hf dots) + bk*sum(q0)
            s5 = vecp.tile([C, 5], F32, tag="s5")
            nc.vector.tensor_reduce(
                s5[:], st["dots"][:].rearrange("c (m h) -> c m h", m=5),
                mybir.AxisListType.X, alu.add)
            nc.vector.tensor_tensor(s5[:], s5[:], mscale, alu.mult)
            q0sum = vecp.tile([C, 1], F32, tag="v")
            nc.vector.tensor_reduce(q0sum[:], st["q0s"][:],
                                    mybir.AxisListType.X, alu.add)
            bkq = vecp.tile([C, 1], F32, tag="v")
            nc.vector.tensor_tensor(bkq[:], bk, q0sum[:], alu.mult)
            nc.vector.tensor_tensor(s5[:], s5[:],
                                    bkq[:, 0:1].broadcast_to((C, 5)), alu.add)
            mx = vecp.tile([C, 1], F32, tag="v")
            nc.vector.tensor_reduce(mx[:], s5[:], mybir.AxisListType.X, alu.max)
            nmx = vecp.tile([C, 1], F32, tag="v")
            nc.vector.tensor_scalar_mul(nmx[:], mx[:], -INV_SQRT_C)
            e5 = vecp.tile([C, 5], F32, tag="s5")
            nc.scalar.activation(e5[:], s5[:], actf.Exp, bias=nmx[:, 0:1],
                                 scale=INV_SQRT_C)
            ssum = vecp.tile([C, 1], F32, tag="v")
            nc.vector.tensor_reduce(ssum[:], e5[:], mybir.AxisListType.X,
                                    alu.add)
            sinv = vecp.tile([C, 1], F32, tag="v")
            nc.vector.reciprocal(sinv[:], ssum[:])
            a5 = vecp.tile([C, 5], F32, tag="s5")
            nc.vector.tensor_scalar_mul(a5[:], e5[:], sinv[:, 0:1])

            # vts_m = Wv^T diag(a_m) Pw^T as fp8 value+residual pair tiles:
            # vt0 = [S_V/S_T * vts0 ; its fp8 residual]
            # vt12/vt34 = [S_V*vts_m ; S_V*vts_m'] (m pairs), vtr12/vtr34 =
            # matching fp8 residuals. All for DoubleRow out matmuls.
            names = ["vt0", "vt12", "vt34", "vtr12", "vtr34"]
            tiles = {}
            for nmv in names:
                tiles[nmv] = vtp.tile([C, 2, C], F8, tag="vt", name=f"{nmv}_{img}")
            slots = []
            for m in range(5):
                if m == 0:
                    slots.append((S_V / S_T, tiles["vt0"][:, 0],
                                  tiles["vt0"][:, 1]))
                else:
                    main = "vt12" if m <= 2 else "vt34"
                    resid = "vtr12" if m <= 2 else "vtr34"
                    h = (m - 1) % 2
                    slots.append((S_V, tiles[main][:, h], tiles[resid][:, h]))
            # stage-batched so em (DVE) / vp (PE) / main+resid pipeline
            ems, vps = [], []
            for m in range(5):
                em = emp.tile([C, C], BF16, tag="em")
                nc.vector.tensor_scalar_mul(em[:], pwT[:], a5[:, m:m + 1])
                ems.append(em)
            for m in range(5):
                vp = ppw.tile([C, C], F32, tag="w")
                nc.tensor.matmul(vp[:], wv_b16[:], ems[m][:], start=True,
                                 stop=True)
                vps.append(vp)
                sc8, mt, rt = slots[m]
                if img == 0:
                    nc.vector.tensor_scalar_mul(mt, vp[:], sc8)
                else:
                    nc.scalar.activation(mt, vp[:], actf.Identity,
                                         bias=0.0, scale=sc8)
                nc.vector.scalar_tensor_tensor(rt, vp[:], sc8, mt,
                                               alu.mult, alu.subtract)
            st["vts"] = tiles

        def phase_b(img, c0=0, c1=8, wide=False):
            st = imgs[img]
            v = st["vts"]
            oi = out_d[img].rearrange("c h w -> c (h w)")

            def out_mms(ps, ch):
                c0_ = ch * 512
                mm = nc.tensor.matmul
                mm(ps, v["vt0"][:], dup2(st["t08"][:, c0_:c0_ + 512]),
                   start=True, stop=False, perf_mode=DR)
                mm(ps, v["vt12"][:], tokpair(st, "x8", 0, ch),
                   start=False, stop=False, perf_mode=DR)
                mm(ps, v["vt34"][:], tokpair(st, "x8", 1, ch),
                   start=False, stop=False, perf_mode=DR)
                mm(ps, v["vtr12"][:], tokpair(st, "x8", 0, ch),
                   start=False, stop=False, perf_mode=DR)
                mm(ps, v["vtr34"][:], tokpair(st, "x8", 1, ch),
                   start=False, stop=False, perf_mode=DR)
                mm(ps, v["vt12"][:], tokpair(st, "r8", 0, ch),
                   start=False, stop=False, perf_mode=DR)
                mm(ps, v["vt34"][:], tokpair(st, "r8", 1, ch),
                   start=False, stop=True, perf_mode=DR)

            if wide:
                # B1 uses the ppk banks (free after the last k matmul) for a
                # deeper psum rotation; copies/DMAs stay at 512 granularity
                for cp in range(c0 // 2, c1 // 2):
                    ps = ppk.tile([C, 1024], F32, name="kp")
                    for h in range(2):
                        out_mms(ps[:, h * 512:(h + 1) * 512], cp * 2 + h)
                    for h in range(2):
                        ch = cp * 2 + h
                        oc = outp.tile([C, 512], F32, tag="oc")
                        nc.scalar.activation(oc[:], ps[:, h * 512:(h + 1) * 512],
                                             actf.Identity, bias=cb[:, 0:1],
                                             scale=1.0 / S_V)
                        nc.sync.dma_start(oi[:, ch * 512:(ch + 1) * 512], oc[:])
                return
            for ch in range(c0, c1):
                ps = pp512.tile([C, 512], F32, tag="ps")
                out_mms(ps[:], ch)
                oc = outp.tile([C, 512], F32, tag="oc")
                nc.scalar.activation(oc[:], ps[:], actf.Identity,
                                     bias=cb[:, 0:1], scale=1.0 / S_V)
                nc.sync.dma_start(oi[:, ch * 512:(ch + 1) * 512], oc[:])

        def stage_dump(img):
            st = imgs[img]
            oi = out_d[img].rearrange("c h w -> c (h w)")
            tmp = q0p.tile([C, L], F32, tag="dump")
            if KSTAGE == 2:
                nc.scalar.activation(tmp[:], st["t08"][:], actf.Identity,
                                     bias=0.0, scale=1.0 / S_T)
                nc.sync.dma_start(oi, tmp[:])
            elif KSTAGE == 4:
                nc.sync.dma_start(oi, st["q0"][:])

        if KSTAGE >= 9:
            # global software pipeline over 16 ingest bands; conv half hf
            # of an image needs that image's bands <= 2*hf+2; k/dots of an
            # hf issue after the next hf's conv so dot drains hide under it
            for s in range(3):
                ingest_band(0, s, sub=True)
            conv_q0(0, 0); ingest_band(0, 3); ingest_band(0, 4)
            conv_q0(0, 1); k_dots(0, 0); ingest_band(0, 5); ingest_band(0, 6)
            conv_q0(0, 2); k_dots(0, 1); ingest_band(0, 7); ingest_band(1, 0)
            conv_q0(0, 3); k_dots(0, 2); ingest_band(1, 1); ingest_band(1, 2)
            k_dots(0, 3)
            conv_q0(1, 0); ingest_band(1, 3); ingest_band(1, 4)
            conv_q0(1, 1); k_dots(1, 0); ingest_band(1, 5); ingest_band(1, 6)
            conv_q0(1, 2); k_dots(1, 1); ingest_band(1, 7)
            softmax_vts(0)
            conv_q0(1, 3); k_dots(1, 2)
            phase_b(0, 0, 2)
            k_dots(1, 3)
            phase_b(0, 2, 8)
            softmax_vts(1)
            phase_b(1)
        else:
            for img in range(BPC):
                for s in range(8):
                    ingest_band(img, s)
                for hf in range(4):
                    conv_q0(img, hf)
                for hf in range(4):
                    k_dots(img, hf)
                stage_dump(img)
    return nc


_CACHE = {}


def _get_nc():
    if "nc" not in _CACHE:
        tile_utils.max_sbuf_usage = SBUF_CAP
        nc = bacc.Bacc("TRN2", target_bir_lowering=False, debug=False,
                       num_devices=NCORES)
        build(nc)
        nc.compile()
        _CACHE["nc"] = nc
    return _CACHE["nc"]


def _in_maps(inputs):
    import ml_dtypes
    f8 = ml_dtypes.float8_e4m3
    b16 = ml_dtypes.bfloat16

    gam = np.asarray(inputs["bn_gamma"], np.float32).reshape(C)
    bet = np.asarray(inputs["bn_beta"], np.float32).reshape(C)
    mea = np.asarray(inputs["bn_mean"], np.float32).reshape(C)
    var = np.asarray(inputs["bn_var"], np.float32).reshape(C)
    dww = np.asarray(inputs["dw_w"], np.float32).reshape(C, 49)
    dwb = np.asarray(inputs["dw_b"], np.float32).reshape(C)
    qkv_w = np.asarray(inputs["qkv_w"], np.float32).reshape(3 * C, C)
    qkv_b = np.asarray(inputs["qkv_b"], np.float32).reshape(3 * C)
    pw = np.asarray(inputs["proj_w"], np.float32).reshape(C, C)
    pb = np.asarray(inputs["proj_b"], np.float32).reshape(C)

    bns = gam / np.sqrt(var + EPS)
    bnb = bet - mea * bns
    bq, bk, bv = qkv_b[0:C], qkv_b[C:2 * C], qkv_b[2 * C:3 * C]
    cb = pw @ bv + pb

    vecs = np.zeros((C, 16), np.float32)
    vecs[:, 0] = bns
    vecs[:, 1] = bnb
    vecs[:, 2] = dwb
    vecs[:, 3] = S_T * dwb
    vecs[:, 4] = bq
    vecs[:, 5] = bk
    vecs[:, 6] = cb
    vecs[:, 7] = 1.0 / (S_W * S_T)
    vecs[:, 8:12] = 1.0 / S_W

    w8s = np.zeros((C, 27, 2, C), np.float32)
    wqT = qkv_w[0:C].T          # [j, cq]
    wkT = qkv_w[C:2 * C].T
    w8s[:, 0, 0] = (S_W / 2) * wqT
    w8s[:, 0, 1] = (S_W / 2) * wqT
    w8s[:, 1, 0] = (S_W / 2) * wkT
    w8s[:, 1, 1] = (S_W / 2) * wkT
    ii = np.arange(C)
    for i, (ta, tb, delta) in enumerate(make_pairs()):
        _, khA, kwA = ta
        if tb is None:
            w8s[ii, 2 + i, 0, ii] = (S_W / 2) * dww[:, khA * 7 + kwA]
            w8s[ii, 2 + i, 1, ii] = (S_W / 2) * dww[:, khA * 7 + kwA]
        else:
            _, khB, kwB = tb
            w8s[ii, 2 + i, 0, ii] = S_W * dww[:, khA * 7 + kwA]
            w8s[ii, 2 + i, 1, ii] = S_W * dww[:, khB * 7 + kwB]

    shared = {
        "vecs": vecs,
        "w8s": w8s.astype(f8),
        "wvb": qkv_w[2 * C:3 * C].astype(b16),
        "pwt": np.ascontiguousarray(pw.T),
    }
    xf = np.ascontiguousarray(np.asarray(inputs["x"], np.float32))
    return [dict(shared, x=xf[i * BPC:(i + 1) * BPC]) for i in range(NCORES)]


def kernel(x, bn_gamma, bn_beta, bn_mean, bn_var, dw_w, dw_b, qkv_w, qkv_b,
           proj_w, proj_b):
    nc = _get_nc()
    in_maps = _in_maps(dict(
        x=x, bn_gamma=bn_gamma, bn_beta=bn_beta, bn_mean=bn_mean,
        bn_var=bn_var, dw_w=dw_w, dw_b=dw_b, qkv_w=qkv_w, qkv_b=qkv_b,
        proj_w=proj_w, proj_b=proj_b))
    res = bass_utils.run_bass_kernel_spmd(nc, in_maps,
                                          core_ids=list(range(NCORES)))
    return np.concatenate([r["out"] for r in res.results], axis=0)
